# revision 16
# baseline (speedup 1.0000x reference)
"""Trainium2 Bass kernel for nn_MultiHeadAttention_59614146068609.

Sharding: 8 cores = 2 batches x 4 head-groups (4 heads each).
Each core projects q/k/v for its batch with its head-slice of Wq/Wk/Wv
(column-sharded), runs causal+padded attention for its 4 heads, and
applies its row-slice of Wo, producing a partial [D, S] output. The host
sums the 4 partials per batch and adds bo.

All matmuls run as float32r (2 cycles/row PE mode, ~1e-4 rel err).
Layout trick: scores are computed transposed (S.T[k, q], k on
partitions) so softmax sums come from an appended ones-column of V and
no on-chip transposes are needed anywhere.

The kernel is specialized at build time on kb_cap = number of 128-wide
key blocks that contain any unpadded key (derived from the runtime
key_padding_mask); fully padded key blocks contribute exactly zero
attention weight, so their projection/QK/exp/PV work is skipped.
"""

import numpy as np

S = 2048
B = 2
D = 1024
H = 16
DK = 64
N_CORES = 8
GROUPS = N_CORES // B          # head groups per batch = 4
HPG = H // GROUPS              # heads per group = 4
OC = HPG * DK                  # per-core projected dim = 256
OT = OC // 128                 # o-tiles per core = 2
IT = D // 128                  # contraction tiles = 8
SC = S // 512                  # sequence chunks of 512 = 4
KB = S // 128                  # k blocks of 128 = 16
NEG = -1e30

_cache = {}


def _build_nc(kb_cap):
    import concourse.bacc as bacc
    import concourse.bass as bass
    import concourse.mybir as mybir
    import concourse.tile as tile
    from concourse import library_config

    F32 = mybir.dt.float32
    F32R = mybir.dt.float32r
    FP16 = mybir.dt.float16
    Exp = mybir.ActivationFunctionType.Exp
    Identity = mybir.ActivationFunctionType.Identity
    PSUM = bass.MemorySpace.PSUM

    ksc = -(-kb_cap * 128 // 512)        # 512-chunks of k_T to project
    vrounds = [
        range(r * 8, min(kb_cap, (r + 1) * 8)) for r in range(-(-kb_cap // 8))
    ]

    nc = bacc.Bacc("TRN2", target_bir_lowering=False, debug=False)

    xq = nc.dram_tensor("xq", [D, S], F32R, kind="ExternalInput")
    xk = nc.dram_tensor("xk", [D, S], F32R, kind="ExternalInput")
    xv = nc.dram_tensor("xv", [D, S], F32R, kind="ExternalInput")
    wq = nc.dram_tensor("wq", [D, OC], F32R, kind="ExternalInput")
    wk = nc.dram_tensor("wk", [D, OC], F32R, kind="ExternalInput")
    wv = nc.dram_tensor("wv", [D, OC], F32R, kind="ExternalInput")
    wo = nc.dram_tensor("wo", [OC, D], FP16, kind="ExternalInput")
    bias_qk = nc.dram_tensor("bias_qk", [128, 2, OT], F32, kind="ExternalInput")
    bias_v = nc.dram_tensor("bias_v", [1, OC], F32, kind="ExternalInput")
    pad = nc.dram_tensor("pad", [128, KB], F32, kind="ExternalInput")
    causal = nc.dram_tensor("causal", [128, 128], F32, kind="ExternalInput")
    out_t = nc.dram_tensor("out_t", [D, S], F32, kind="ExternalOutput")

    with tile.TileContext(nc) as tc, nc.allow_low_precision(
        reason="fp32r compute throughout; validated vs fp64 reference"
    ):
        with (
            tc.tile_pool(name="persist", bufs=1) as pp,
            tc.tile_pool(name="xs", bufs=6) as xs,
        ):
            nc.gpsimd.load_library(library_config.attn)

            # ---- persistent SBUF tensors ----
            t_wq = pp.tile([128, IT, OC], F32R)
            t_wk = pp.tile([128, IT, OC], F32R)
            t_wv = pp.tile([128, IT, OC], F32R)
            t_wo = pp.tile([128, OT, D], FP16)
            t_bqk = pp.tile([128, 2, OT], F32)
            t_bv = pp.tile([128, OC], F32)
            t_pad = pp.tile([128, KB], F32)
            t_causal = pp.tile([128, 128], F32)
            t_qT = pp.tile([128, HPG, S], F32R)
            t_kT = pp.tile([128, HPG, ksc * 512], F32R)
            t_V = pp.tile([128, kb_cap, HPG, 128], FP16)
            t_OT = pp.tile([128, OT, S], FP16)

            nc.scalar.dma_start(out=t_wq, in_=wq[:].rearrange("(i p) o -> p i o", p=128))
            nc.scalar.dma_start(out=t_wk, in_=wk[:].rearrange("(i p) o -> p i o", p=128))
            nc.scalar.dma_start(out=t_wv, in_=wv[:].rearrange("(i p) o -> p i o", p=128))
            nc.scalar.dma_start(out=t_bqk, in_=bias_qk[:])
            nc.scalar.dma_start(out=t_pad, in_=pad[:])
            nc.scalar.dma_start(out=t_causal, in_=causal[:])
            # broadcast the v bias across partitions once (free dim = o)
            t_bv1 = pp.tile([1, OC], F32)
            nc.scalar.dma_start(out=t_bv1, in_=bias_v[:])
            nc.scalar.dma_start(out=t_wo, in_=wo[:].rearrange("(j p) d -> p j d", p=128))
            nc.gpsimd.partition_broadcast(t_bv, t_bv1)
            nc.gpsimd.memset(t_qT[64:128, :, :].bitcast(F32), 0)
            nc.gpsimd.memset(t_kT[64:128, :, :].bitcast(F32), 0)
            nc.gpsimd.memset(t_V[:], 0)
            nc.vector.memset(t_V[:, :, :, DK : DK + 1], 1.0)

            # ---- phase A: projections ----
            # q and k land transposed ([o, s], o on partitions); v lands
            # natural ([s, o], s on partitions) for the PV matmul.
            with tc.tile_pool(name="ps_proj", bufs=8, space=PSUM) as ps_proj:
                for name, xin, w_sb, nsc in (("q", xq, t_wq, SC), ("k", xk, t_wk, ksc)):
                    dst = t_qT if name == "q" else t_kT
                    bidx = 0 if name == "q" else 1
                    acc = [
                        ps_proj.tile(
                            [128, 512], F32, tag="proj", name=f"acc_{name}_{n}"
                        )
                        for n in range(OT * nsc)
                    ]
                    for i in range(IT):
                        for sc in range(nsc):
                            xt = xs.tile([128, 512], F32R, tag="x", name=f"xt_{name}_{i}_{sc}", bufs=8)
                            nc.sync.dma_start(
                                out=xt,
                                in_=xin[
                                    i * 128 : (i + 1) * 128, sc * 512 : (sc + 1) * 512
                                ],
                            )
                            for ot in range(OT):
                                nc.tensor.matmul(
                                    acc[ot * nsc + sc],
                                    w_sb[:, i, ot * 128 : (ot + 1) * 128],
                                    xt,
                                    start=(i == 0),
                                    stop=(i == IT - 1),
                                )
                    for ot in range(OT):
                        for sc in range(nsc):
                            for half in range(2):
                                h = 2 * ot + half
                                p0 = half * 64
                                nc.scalar.activation(
                                    out=dst[0:64, h, sc * 512 : (sc + 1) * 512],
                                    in_=acc[ot * nsc + sc][p0 : p0 + 64, :],
                                    func=Identity,
                                    bias=t_bqk[p0 : p0 + 64, bidx, ot : ot + 1],
                                    scale=1.0,
                                )

                # v natural: lhsT = x tile (stationary), rhs = wv (moving).
                # One accumulation group per psum bank (interleaving two
                # start/accumulate groups in one bank corrupts has_written).
                for rnd, sts in enumerate(vrounds):
                    sts = list(sts)
                    w = len(sts) * 128
                    vacc = [
                        ps_proj.tile([128, OC], F32, tag="proj", name=f"vacc_{rnd}_{n}")
                        for n in range(len(sts))
                    ]
                    for i in range(IT):
                        xt = xs.tile([128, w], F32R, tag="xv", name=f"xtv_{rnd}_{i}", bufs=3)
                        nc.sync.dma_start(
                            out=xt,
                            in_=xv[
                                i * 128 : (i + 1) * 128,
                                sts[0] * 128 : sts[0] * 128 + w,
                            ],
                        )
                        for n in range(len(sts)):
                            nc.tensor.matmul(
                                vacc[n],
                                xt[:, n * 128 : (n + 1) * 128],
                                t_wv[:, i, :],
                                start=(i == 0),
                                stop=(i == IT - 1),
                            )
                    for n, st in enumerate(sts):
                        nc.vector.tensor_add(
                            out=t_V[:, st, :, 0:DK],
                            in0=vacc[n].rearrange("p (h d) -> p h d", h=HPG),
                            in1=t_bv.rearrange("p (h d) -> p h d", h=HPG),
                        )

            # ---- phase B: attention (S.T layout) + interleaved phase C ----
            with (
                tc.tile_pool(name="ps_att", bufs=3, space=PSUM) as ps_att,
                tc.tile_pool(name="ps_o", bufs=3, space=PSUM) as ps_o,
                tc.tile_pool(name="ps_c", bufs=2, space=PSUM) as ps_c,
                tc.tile_pool(name="pb", bufs=4) as pb,
                tc.tile_pool(name="nrm", bufs=2) as nrm,
                tc.tile_pool(name="stg", bufs=3) as stg,
            ):
                for qc in range(SC):
                    q0 = qc * 512
                    nkb = min(4 * (qc + 1), kb_cap)
                    for pair in ((0, 1), (2, 3)):
                        o_ps = {
                            h: ps_o.tile(
                                [128, 512], F32, tag="ops", name=f"ops_{qc}_{h}"
                            )
                            for h in pair
                        }
                        for kb in range(nkb):
                            k0 = kb * 128
                            off = max(0, k0 - q0)
                            st = {}
                            # adjacent K=64 matmuls at row groups 0 / 64 can
                            # run concurrently in the PE array
                            for h in pair:
                                st[h] = ps_att.tile(
                                    [128, 512], F32, tag="st", name=f"st_{qc}_{h}_{kb}"
                                )
                                nc.tensor.matmul(
                                    st[h][:, off:512],
                                    t_kT[:, h, k0 : k0 + 128],
                                    t_qT[:, h, q0 + off : q0 + 512],
                                    start=True,
                                    stop=True,
                                )
                            for h in pair:
                                if k0 >= q0:
                                    nc.vector.tensor_add(
                                        out=st[h][:, off : off + 128],
                                        in0=st[h][:, off : off + 128],
                                        in1=t_causal,
                                    )
                                pt = pb.tile(
                                    [128, 512], FP16, tag="pt", name=f"pt_{qc}_{h}_{kb}"
                                )
                                nc.scalar.activation(
                                    out=pt[:, off:512],
                                    in_=st[h][:, off:512],
                                    func=Exp,
                                    bias=t_pad[:, kb : kb + 1],
                                    scale=1.0,
                                )
                                nc.tensor.matmul(
                                    o_ps[h][:, off:512],
                                    t_V[:, kb, h, :],
                                    pt[:, off:512],
                                    start=(kb == 0),
                                    stop=(kb == nkb - 1),
                                )
                        for h in pair:
                            ot, p0 = h // 2, (h % 2) * 64
                            t_l = nrm.tile([128, 512], F32, tag="l", name=f"l_{qc}_{h}")
                            nc.scalar.copy(t_l[0:1, :], o_ps[h][DK : DK + 1, :])
                            t_r = nrm.tile([128, 512], F32, tag="r", name=f"r_{qc}_{h}")
                            nc.vector.reciprocal_approx_fast(t_r[0:1, :], t_l[0:1, :])
                            t_rb = nrm.tile([DK, 512], F32, tag="rb", name=f"rb_{qc}_{h}")
                            nc.gpsimd.partition_broadcast(t_rb, t_r[0:1, :])
                            nc.vector.tensor_mul(
                                t_OT[p0 : p0 + DK, ot, q0 : q0 + 512],
                                o_ps[h][0:DK, :],
                                t_rb,
                            )
                    # phase C for this 512-chunk of s (needs all 4 heads)
                    for dt_ in range(D // 128):
                        ops = ps_c.tile([128, 512], F32, tag="c", name=f"c_{qc}_{dt_}")
                        for j in range(OT):
                            nc.tensor.matmul(
                                ops,
                                t_wo[:, j, dt_ * 128 : (dt_ + 1) * 128],
                                t_OT[:, j, q0 : q0 + 512],
                                start=(j == 0),
                                stop=(j == OT - 1),
                            )
                        st_o = stg.tile([128, 512], F32, tag="s", name=f"so_{qc}_{dt_}")
                        nc.vector.tensor_copy(st_o, ops)
                        nc.sync.dma_start(
                            out=out_t[dt_ * 128 : (dt_ + 1) * 128, q0 : q0 + 512],
                            in_=st_o,
                        )
    nc.compile()
    return nc


def _get_nc(kb_cap):
    key = ("nc", kb_cap)
    if key not in _cache:
        _cache[key] = _build_nc(kb_cap)
    return _cache[key]


def kernel(
    query,
    key,
    value,
    Wq,
    bq,
    Wk,
    bk,
    Wv,
    bv,
    Wo,
    bo,
    attn_mask,
    key_padding_mask,
):
    import ml_dtypes
    from concourse import bass_utils

    query = np.asarray(query, dtype=np.float32)
    key = np.asarray(key, dtype=np.float32)
    value = np.asarray(value, dtype=np.float32)
    Wq = np.asarray(Wq, dtype=np.float32)
    bq = np.asarray(bq, dtype=np.float32)
    Wk = np.asarray(Wk, dtype=np.float32)
    bk = np.asarray(bk, dtype=np.float32)
    Wv = np.asarray(Wv, dtype=np.float32)
    bv = np.asarray(bv, dtype=np.float32)
    Wo = np.asarray(Wo, dtype=np.float32)
    bo = np.asarray(bo, dtype=np.float32)
    attn_mask = np.asarray(attn_mask)
    key_padding_mask = np.asarray(key_padding_mask)

    # this kernel hardcodes the causal structure of attn_mask
    expected = np.triu(np.ones((S, S), dtype=bool), k=1)
    assert np.array_equal(attn_mask, expected), "kernel assumes causal attn_mask"

    # number of 128-blocks that contain any valid (unpadded) key
    valid = ~key_padding_mask  # [B, S]
    kb_cap = 0
    for b in range(B):
        nz = np.nonzero(valid[b])[0]
        cap = (int(nz.max()) // 128 + 1) if nz.size else 1
        kb_cap = max(kb_cap, cap)

    scale = np.float32(1.0 / np.sqrt(DK))
    causal_tile = np.where(
        np.arange(128)[None, :] >= np.arange(128)[:, None], 0.0, NEG
    ).astype(np.float32)

    # per-batch transposed activations (shared by the batch's 4 cores)
    xq_b = [np.ascontiguousarray(query[:, b, :].T) for b in range(B)]
    xk_b = [np.ascontiguousarray(key[:, b, :].T) for b in range(B)]
    xv_b = [np.ascontiguousarray(value[:, b, :].T) for b in range(B)]
    pad_b = [
        np.ascontiguousarray(
            np.where(key_padding_mask[b], NEG, 0.0)
            .astype(np.float32)
            .reshape(KB, 128)
            .T
        )
        for b in range(B)
    ]

    in_maps = []
    for c in range(N_CORES):
        b = c // GROUPS
        g = c % GROUPS
        o0 = g * OC
        osl = slice(o0, o0 + OC)
        bias_qk = np.stack(
            [
                (bq[osl] * scale).reshape(OT, 128).T,
                bk[osl].reshape(OT, 128).T,
            ],
            axis=1,
        ).astype(np.float32)  # [128, 2, OT]
        in_maps.append(
            {
                "xq": xq_b[b],
                "xk": xk_b[b],
                "xv": xv_b[b],
                "wq": np.ascontiguousarray((Wq[osl, :] * scale).T),
                "wk": np.ascontiguousarray(Wk[osl, :].T),
                "wv": np.ascontiguousarray(Wv[osl, :].T),
                "wo": np.ascontiguousarray(Wo[:, osl].T).astype(np.float16),
                "bias_qk": np.ascontiguousarray(bias_qk),
                "bias_v": np.ascontiguousarray(bv[osl][None, :]),
                "pad": pad_b[b],
                "causal": causal_tile,
            }
        )

    res = bass_utils.run_bass_kernel_spmd(
        _get_nc(kb_cap), in_maps, core_ids=list(range(N_CORES))
    )
    _cache["last_res"] = res

    out = np.zeros((S, B, D), dtype=np.float32)
    for b in range(B):
        acc = np.zeros((D, S), dtype=np.float32)
        for g in range(GROUPS):
            acc += res.results[b * GROUPS + g]["out_t"]
        out[:, b, :] = acc.T + bo[None, :]
    return out


# revision 17
# speedup vs baseline: 1.1251x; 1.1251x over previous
"""Trainium2 Bass kernel for nn_MultiHeadAttention_59614146068609.

Sharding: 8 cores = 2 batches x 4 head-groups (4 heads each).
Each core projects q/k/v for its batch with its head-slice of Wq/Wk/Wv
(column-sharded), runs causal+padded attention for its 4 heads, and
applies its row-slice of Wo, producing a partial [D, S] output. The host
sums the 4 partials per batch and adds bo.

All matmuls run as float32r (2 cycles/row PE mode, ~1e-4 rel err).
Layout trick: scores are computed transposed (S.T[k, q], k on
partitions) so softmax sums come from an appended ones-column of V and
no on-chip transposes are needed anywhere.

The kernel is specialized at build time on kb_cap = number of 128-wide
key blocks that contain any unpadded key (derived from the runtime
key_padding_mask); fully padded key blocks contribute exactly zero
attention weight, so their projection/QK/exp/PV work is skipped.
"""

import numpy as np

S = 2048
B = 2
D = 1024
H = 16
DK = 64
N_CORES = 8
GROUPS = N_CORES // B          # head groups per batch = 4
HPG = H // GROUPS              # heads per group = 4
OC = HPG * DK                  # per-core projected dim = 256
OT = OC // 128                 # o-tiles per core = 2
IT = D // 128                  # contraction tiles = 8
SC = S // 512                  # sequence chunks of 512 = 4
KB = S // 128                  # k blocks of 128 = 16
NEG = -1e30

_cache = {}


def _build_nc(kb_cap):
    import concourse.bacc as bacc
    import concourse.bass as bass
    import concourse.mybir as mybir
    import concourse.tile as tile
    from concourse import library_config

    F32 = mybir.dt.float32
    F32R = mybir.dt.float32r
    FP16 = mybir.dt.float16
    Exp = mybir.ActivationFunctionType.Exp
    Identity = mybir.ActivationFunctionType.Identity
    PSUM = bass.MemorySpace.PSUM

    ksc = -(-kb_cap * 128 // 512)        # 512-chunks of k_T to project
    vrounds = [
        range(r * 8, min(kb_cap, (r + 1) * 8)) for r in range(-(-kb_cap // 8))
    ]

    nc = bacc.Bacc("TRN2", target_bir_lowering=False, debug=False)

    xq = nc.dram_tensor("xq", [D, S], FP16, kind="ExternalInput")
    xk = nc.dram_tensor("xk", [D, S], FP16, kind="ExternalInput")
    xv = nc.dram_tensor("xv", [D, S], FP16, kind="ExternalInput")
    wq = nc.dram_tensor("wq", [D, OC], FP16, kind="ExternalInput")
    wk = nc.dram_tensor("wk", [D, OC], FP16, kind="ExternalInput")
    wv = nc.dram_tensor("wv", [D, OC], FP16, kind="ExternalInput")
    wo = nc.dram_tensor("wo", [OC, D], FP16, kind="ExternalInput")
    bias_qk = nc.dram_tensor("bias_qk", [128, 2, OT], F32, kind="ExternalInput")
    bias_v = nc.dram_tensor("bias_v", [1, OC], F32, kind="ExternalInput")
    pad = nc.dram_tensor("pad", [128, KB], F32, kind="ExternalInput")
    causal = nc.dram_tensor("causal", [128, 128], F32, kind="ExternalInput")
    out_t = nc.dram_tensor("out_t", [D, S], F32, kind="ExternalOutput")

    with tile.TileContext(nc) as tc, nc.allow_low_precision(
        reason="fp32r compute throughout; validated vs fp64 reference"
    ):
        with (
            tc.tile_pool(name="persist", bufs=1) as pp,
            tc.tile_pool(name="xs", bufs=6) as xs,
        ):
            nc.gpsimd.load_library(library_config.attn)

            # ---- persistent SBUF tensors ----
            t_wq = pp.tile([128, IT, OC], FP16)
            t_wk = pp.tile([128, IT, OC], FP16)
            t_wv = pp.tile([128, IT, OC], FP16)
            t_wo = pp.tile([128, OT, D], FP16)
            t_bqk = pp.tile([128, 2, OT], F32)
            t_bv = pp.tile([128, OC], F32)
            t_pad = pp.tile([128, KB], F32)
            t_causal = pp.tile([128, 128], F32)
            t_qT = pp.tile([128, HPG, S], FP16)
            t_kT = pp.tile([128, HPG, ksc * 512], FP16)
            t_V = pp.tile([128, kb_cap, HPG, 128], FP16)
            t_OT = pp.tile([128, OT, S], FP16)

            nc.scalar.dma_start(out=t_wq, in_=wq[:].rearrange("(i p) o -> p i o", p=128))
            nc.scalar.dma_start(out=t_wk, in_=wk[:].rearrange("(i p) o -> p i o", p=128))
            nc.scalar.dma_start(out=t_wv, in_=wv[:].rearrange("(i p) o -> p i o", p=128))
            nc.scalar.dma_start(out=t_bqk, in_=bias_qk[:])
            nc.scalar.dma_start(out=t_pad, in_=pad[:])
            nc.scalar.dma_start(out=t_causal, in_=causal[:])
            # broadcast the v bias across partitions once (free dim = o)
            t_bv1 = pp.tile([1, OC], F32)
            nc.scalar.dma_start(out=t_bv1, in_=bias_v[:])
            nc.scalar.dma_start(out=t_wo, in_=wo[:].rearrange("(j p) d -> p j d", p=128))
            nc.gpsimd.partition_broadcast(t_bv, t_bv1)
            nc.gpsimd.memset(t_qT[64:128, :, :], 0)
            nc.gpsimd.memset(t_kT[64:128, :, :], 0)
            nc.gpsimd.memset(t_V[:], 0)
            nc.vector.memset(t_V[:, :, :, DK : DK + 1], 1.0)

            # ---- phase A: projections ----
            # q and k land transposed ([o, s], o on partitions); v lands
            # natural ([s, o], s on partitions) for the PV matmul.
            with tc.tile_pool(name="ps_proj", bufs=8, space=PSUM) as ps_proj:
                for name, xin, w_sb, nsc in (("q", xq, t_wq, SC), ("k", xk, t_wk, ksc)):
                    dst = t_qT if name == "q" else t_kT
                    bidx = 0 if name == "q" else 1
                    acc = [
                        ps_proj.tile(
                            [128, 512], F32, tag="proj", name=f"acc_{name}_{n}"
                        )
                        for n in range(OT * nsc)
                    ]
                    for i in range(IT):
                        for sc in range(nsc):
                            xt = xs.tile([128, 512], FP16, tag="x", name=f"xt_{name}_{i}_{sc}", bufs=8)
                            nc.sync.dma_start(
                                out=xt,
                                in_=xin[
                                    i * 128 : (i + 1) * 128, sc * 512 : (sc + 1) * 512
                                ],
                            )
                            for ot in range(OT):
                                nc.tensor.matmul(
                                    acc[ot * nsc + sc],
                                    w_sb[:, i, ot * 128 : (ot + 1) * 128],
                                    xt,
                                    start=(i == 0),
                                    stop=(i == IT - 1),
                                )
                    for ot in range(OT):
                        for sc in range(nsc):
                            for half in range(2):
                                h = 2 * ot + half
                                p0 = half * 64
                                nc.scalar.activation(
                                    out=dst[0:64, h, sc * 512 : (sc + 1) * 512],
                                    in_=acc[ot * nsc + sc][p0 : p0 + 64, :],
                                    func=Identity,
                                    bias=t_bqk[p0 : p0 + 64, bidx, ot : ot + 1],
                                    scale=1.0,
                                )

                # v natural: lhsT = x tile (stationary), rhs = wv (moving).
                # One accumulation group per psum bank (interleaving two
                # start/accumulate groups in one bank corrupts has_written).
                for rnd, sts in enumerate(vrounds):
                    sts = list(sts)
                    w = len(sts) * 128
                    vacc = [
                        ps_proj.tile([128, OC], F32, tag="proj", name=f"vacc_{rnd}_{n}")
                        for n in range(len(sts))
                    ]
                    for i in range(IT):
                        xt = xs.tile([128, w], FP16, tag="xv", name=f"xtv_{rnd}_{i}", bufs=3)
                        nc.sync.dma_start(
                            out=xt,
                            in_=xv[
                                i * 128 : (i + 1) * 128,
                                sts[0] * 128 : sts[0] * 128 + w,
                            ],
                        )
                        for n in range(len(sts)):
                            nc.tensor.matmul(
                                vacc[n],
                                xt[:, n * 128 : (n + 1) * 128],
                                t_wv[:, i, :],
                                start=(i == 0),
                                stop=(i == IT - 1),
                            )
                    for n, st in enumerate(sts):
                        nc.vector.tensor_add(
                            out=t_V[:, st, :, 0:DK],
                            in0=vacc[n].rearrange("p (h d) -> p h d", h=HPG),
                            in1=t_bv.rearrange("p (h d) -> p h d", h=HPG),
                        )

            # ---- phase B: attention (S.T layout) + interleaved phase C ----
            with (
                tc.tile_pool(name="ps_att", bufs=3, space=PSUM) as ps_att,
                tc.tile_pool(name="ps_o", bufs=3, space=PSUM) as ps_o,
                tc.tile_pool(name="ps_c", bufs=2, space=PSUM) as ps_c,
                tc.tile_pool(name="pb", bufs=4) as pb,
                tc.tile_pool(name="nrm", bufs=2) as nrm,
                tc.tile_pool(name="stg", bufs=3) as stg,
            ):
                for qc in range(SC):
                    q0 = qc * 512
                    nkb = min(4 * (qc + 1), kb_cap)
                    for pair in ((0, 1), (2, 3)):
                        o_ps = {
                            h: ps_o.tile(
                                [128, 512], F32, tag="ops", name=f"ops_{qc}_{h}"
                            )
                            for h in pair
                        }
                        for kb in range(nkb):
                            k0 = kb * 128
                            off = max(0, k0 - q0)
                            st = {}
                            # adjacent K=64 matmuls at row groups 0 / 64 can
                            # run concurrently in the PE array
                            for h in pair:
                                st[h] = ps_att.tile(
                                    [128, 512], F32, tag="st", name=f"st_{qc}_{h}_{kb}"
                                )
                                nc.tensor.matmul(
                                    st[h][:, off:512],
                                    t_kT[:, h, k0 : k0 + 128],
                                    t_qT[:, h, q0 + off : q0 + 512],
                                    start=True,
                                    stop=True,
                                )
                            for h in pair:
                                if k0 >= q0:
                                    nc.vector.tensor_add(
                                        out=st[h][:, off : off + 128],
                                        in0=st[h][:, off : off + 128],
                                        in1=t_causal,
                                    )
                                pt = pb.tile(
                                    [128, 512], FP16, tag="pt", name=f"pt_{qc}_{h}_{kb}"
                                )
                                nc.scalar.activation(
                                    out=pt[:, off:512],
                                    in_=st[h][:, off:512],
                                    func=Exp,
                                    bias=t_pad[:, kb : kb + 1],
                                    scale=1.0,
                                )
                                nc.tensor.matmul(
                                    o_ps[h][:, off:512],
                                    t_V[:, kb, h, :],
                                    pt[:, off:512],
                                    start=(kb == 0),
                                    stop=(kb == nkb - 1),
                                )
                        for h in pair:
                            ot, p0 = h // 2, (h % 2) * 64
                            t_l = nrm.tile([128, 512], F32, tag="l", name=f"l_{qc}_{h}")
                            nc.scalar.copy(t_l[0:1, :], o_ps[h][DK : DK + 1, :])
                            t_r = nrm.tile([128, 512], F32, tag="r", name=f"r_{qc}_{h}")
                            nc.vector.reciprocal_approx_fast(t_r[0:1, :], t_l[0:1, :])
                            t_rb = nrm.tile([DK, 512], F32, tag="rb", name=f"rb_{qc}_{h}")
                            nc.gpsimd.partition_broadcast(t_rb, t_r[0:1, :])
                            nc.vector.tensor_mul(
                                t_OT[p0 : p0 + DK, ot, q0 : q0 + 512],
                                o_ps[h][0:DK, :],
                                t_rb,
                            )
                    # phase C for this 512-chunk of s (needs all 4 heads)
                    for dt_ in range(D // 128):
                        ops = ps_c.tile([128, 512], F32, tag="c", name=f"c_{qc}_{dt_}")
                        for j in range(OT):
                            nc.tensor.matmul(
                                ops,
                                t_wo[:, j, dt_ * 128 : (dt_ + 1) * 128],
                                t_OT[:, j, q0 : q0 + 512],
                                start=(j == 0),
                                stop=(j == OT - 1),
                            )
                        st_o = stg.tile([128, 512], F32, tag="s", name=f"so_{qc}_{dt_}")
                        nc.vector.tensor_copy(st_o, ops)
                        nc.sync.dma_start(
                            out=out_t[dt_ * 128 : (dt_ + 1) * 128, q0 : q0 + 512],
                            in_=st_o,
                        )
    nc.compile()
    return nc


def _get_nc(kb_cap):
    key = ("nc", kb_cap)
    if key not in _cache:
        _cache[key] = _build_nc(kb_cap)
    return _cache[key]


def kernel(
    query,
    key,
    value,
    Wq,
    bq,
    Wk,
    bk,
    Wv,
    bv,
    Wo,
    bo,
    attn_mask,
    key_padding_mask,
):
    import ml_dtypes
    from concourse import bass_utils

    query = np.asarray(query, dtype=np.float32)
    key = np.asarray(key, dtype=np.float32)
    value = np.asarray(value, dtype=np.float32)
    Wq = np.asarray(Wq, dtype=np.float32)
    bq = np.asarray(bq, dtype=np.float32)
    Wk = np.asarray(Wk, dtype=np.float32)
    bk = np.asarray(bk, dtype=np.float32)
    Wv = np.asarray(Wv, dtype=np.float32)
    bv = np.asarray(bv, dtype=np.float32)
    Wo = np.asarray(Wo, dtype=np.float32)
    bo = np.asarray(bo, dtype=np.float32)
    attn_mask = np.asarray(attn_mask)
    key_padding_mask = np.asarray(key_padding_mask)

    # this kernel hardcodes the causal structure of attn_mask
    expected = np.triu(np.ones((S, S), dtype=bool), k=1)
    assert np.array_equal(attn_mask, expected), "kernel assumes causal attn_mask"

    # number of 128-blocks that contain any valid (unpadded) key
    valid = ~key_padding_mask  # [B, S]
    kb_cap = 0
    for b in range(B):
        nz = np.nonzero(valid[b])[0]
        cap = (int(nz.max()) // 128 + 1) if nz.size else 1
        kb_cap = max(kb_cap, cap)

    scale = np.float32(1.0 / np.sqrt(DK))
    causal_tile = np.where(
        np.arange(128)[None, :] >= np.arange(128)[:, None], 0.0, NEG
    ).astype(np.float32)

    # per-batch transposed activations (shared by the batch's 4 cores)
    xq_b = [np.ascontiguousarray(query[:, b, :].T.astype(np.float16)) for b in range(B)]
    xk_b = [np.ascontiguousarray(key[:, b, :].T.astype(np.float16)) for b in range(B)]
    xv_b = [np.ascontiguousarray(value[:, b, :].T.astype(np.float16)) for b in range(B)]
    pad_b = [
        np.ascontiguousarray(
            np.where(key_padding_mask[b], NEG, 0.0)
            .astype(np.float32)
            .reshape(KB, 128)
            .T
        )
        for b in range(B)
    ]

    in_maps = []
    for c in range(N_CORES):
        b = c // GROUPS
        g = c % GROUPS
        o0 = g * OC
        osl = slice(o0, o0 + OC)
        bias_qk = np.stack(
            [
                (bq[osl] * scale).reshape(OT, 128).T,
                bk[osl].reshape(OT, 128).T,
            ],
            axis=1,
        ).astype(np.float32)  # [128, 2, OT]
        in_maps.append(
            {
                "xq": xq_b[b],
                "xk": xk_b[b],
                "xv": xv_b[b],
                "wq": np.ascontiguousarray((Wq[osl, :] * scale).T.astype(np.float16)),
                "wk": np.ascontiguousarray(Wk[osl, :].T.astype(np.float16)),
                "wv": np.ascontiguousarray(Wv[osl, :].T.astype(np.float16)),
                "wo": np.ascontiguousarray(Wo[:, osl].T).astype(np.float16),
                "bias_qk": np.ascontiguousarray(bias_qk),
                "bias_v": np.ascontiguousarray(bv[osl][None, :]),
                "pad": pad_b[b],
                "causal": causal_tile,
            }
        )

    res = bass_utils.run_bass_kernel_spmd(
        _get_nc(kb_cap), in_maps, core_ids=list(range(N_CORES))
    )
    _cache["last_res"] = res

    out = np.zeros((S, B, D), dtype=np.float32)
    for b in range(B):
        acc = np.zeros((D, S), dtype=np.float32)
        for g in range(GROUPS):
            acc += res.results[b * GROUPS + g]["out_t"]
        out[:, b, :] = acc.T + bo[None, :]
    return out


# revision 18
# speedup vs baseline: 1.2176x; 1.0822x over previous
"""Trainium2 Bass kernel for nn_MultiHeadAttention_59614146068609.

Sharding: 8 cores = 2 batches x 4 head-groups (4 heads each).
Each core projects q/k/v for its batch with its head-slice of Wq/Wk/Wv
(column-sharded), runs causal+padded attention for its 4 heads, and
applies its row-slice of Wo, producing a partial [D, S] output. The host
sums the 4 partials per batch and adds bo.

All matmuls run as float32r (2 cycles/row PE mode, ~1e-4 rel err).
Layout trick: scores are computed transposed (S.T[k, q], k on
partitions) so softmax sums come from an appended ones-column of V and
no on-chip transposes are needed anywhere.

The kernel is specialized at build time on kb_cap = number of 128-wide
key blocks that contain any unpadded key (derived from the runtime
key_padding_mask); fully padded key blocks contribute exactly zero
attention weight, so their projection/QK/exp/PV work is skipped.
"""

import numpy as np

S = 2048
B = 2
D = 1024
H = 16
DK = 64
N_CORES = 8
GROUPS = N_CORES // B          # head groups per batch = 4
HPG = H // GROUPS              # heads per group = 4
OC = HPG * DK                  # per-core projected dim = 256
OT = OC // 128                 # o-tiles per core = 2
IT = D // 128                  # contraction tiles = 8
SC = S // 512                  # sequence chunks of 512 = 4
KB = S // 128                  # k blocks of 128 = 16
NEG = -1e30

_cache = {}


def _build_nc(kb_cap):
    import concourse.bacc as bacc
    import concourse.bass as bass
    import concourse.mybir as mybir
    import concourse.tile as tile
    from concourse import library_config

    F32 = mybir.dt.float32
    F32R = mybir.dt.float32r
    FP16 = mybir.dt.float16
    Exp = mybir.ActivationFunctionType.Exp
    Identity = mybir.ActivationFunctionType.Identity
    PSUM = bass.MemorySpace.PSUM

    ksc = -(-kb_cap * 128 // 512)        # 512-chunks of k_T to project
    vrounds = [
        range(r * 8, min(kb_cap, (r + 1) * 8)) for r in range(-(-kb_cap // 8))
    ]

    nc = bacc.Bacc("TRN2", target_bir_lowering=False, debug=False)

    xq = nc.dram_tensor("xq", [D, S], FP16, kind="ExternalInput")
    xk = nc.dram_tensor("xk", [D, S], FP16, kind="ExternalInput")
    xv = nc.dram_tensor("xv", [D, S], FP16, kind="ExternalInput")
    wq = nc.dram_tensor("wq", [D, OC], FP16, kind="ExternalInput")
    wk = nc.dram_tensor("wk", [D, OC], FP16, kind="ExternalInput")
    wv = nc.dram_tensor("wv", [D, OC], FP16, kind="ExternalInput")
    wo = nc.dram_tensor("wo", [OC, D], FP16, kind="ExternalInput")
    bias_qk = nc.dram_tensor("bias_qk", [128, 2, OT], F32, kind="ExternalInput")
    bias_v = nc.dram_tensor("bias_v", [1, OC], F32, kind="ExternalInput")
    pad = nc.dram_tensor("pad", [128, KB], F32, kind="ExternalInput")
    causal = nc.dram_tensor("causal", [128, 128], F32, kind="ExternalInput")
    out_t = nc.dram_tensor("out_t", [D, S], F32, kind="ExternalOutput")

    with tile.TileContext(nc) as tc, nc.allow_low_precision(
        reason="fp32r compute throughout; validated vs fp64 reference"
    ):
        with (
            tc.tile_pool(name="persist", bufs=1) as pp,
            tc.tile_pool(name="xs", bufs=6) as xs,
        ):
            nc.gpsimd.load_library(library_config.attn)

            # ---- persistent SBUF tensors ----
            t_wq = pp.tile([128, IT, OC], FP16)
            t_wk = pp.tile([128, IT, OC], FP16)
            t_wv = pp.tile([128, IT, OC], FP16)
            t_wo = pp.tile([128, OT, D], FP16)
            t_bqk = pp.tile([128, 2, OT], F32)
            t_bv = pp.tile([128, OC], F32)
            t_pad = pp.tile([128, KB], F32)
            t_causal = pp.tile([128, 128], F32)
            t_qT = pp.tile([128, HPG, S], FP16)
            t_kT = pp.tile([128, HPG, ksc * 512], FP16)
            t_V = pp.tile([128, kb_cap, HPG, 128], FP16)
            t_OT = pp.tile([128, OT, S], FP16)

            nc.scalar.dma_start(out=t_wq, in_=wq[:].rearrange("(i p) o -> p i o", p=128))
            nc.scalar.dma_start(out=t_wk, in_=wk[:].rearrange("(i p) o -> p i o", p=128))
            nc.scalar.dma_start(out=t_wv, in_=wv[:].rearrange("(i p) o -> p i o", p=128))
            nc.scalar.dma_start(out=t_bqk, in_=bias_qk[:])
            nc.scalar.dma_start(out=t_pad, in_=pad[:])
            nc.scalar.dma_start(out=t_causal, in_=causal[:])
            # broadcast the v bias across partitions once (free dim = o)
            t_bv1 = pp.tile([1, OC], F32)
            nc.scalar.dma_start(out=t_bv1, in_=bias_v[:])
            nc.scalar.dma_start(out=t_wo, in_=wo[:].rearrange("(j p) d -> p j d", p=128))
            nc.gpsimd.partition_broadcast(t_bv, t_bv1)
            nc.gpsimd.memset(t_qT[64:128, :, :], 0)
            nc.gpsimd.memset(t_kT[64:128, :, :], 0)
            nc.gpsimd.memset(t_V[:], 0)
            nc.vector.memset(t_V[:, :, :, DK : DK + 1], 1.0)

            # ---- phase A: projections ----
            # q and k land transposed ([o, s], o on partitions); v lands
            # natural ([s, o], s on partitions) for the PV matmul.
            with tc.tile_pool(name="ps_proj", bufs=8, space=PSUM) as ps_proj:
                for name, xin, w_sb, nsc in (("q", xq, t_wq, SC), ("k", xk, t_wk, ksc)):
                    dst = t_qT if name == "q" else t_kT
                    bidx = 0 if name == "q" else 1
                    acc = [
                        ps_proj.tile(
                            [128, 512], F32, tag="proj", name=f"acc_{name}_{n}"
                        )
                        for n in range(OT * nsc)
                    ]
                    for i in range(IT):
                        xt = xs.tile(
                            [128, nsc * 512], FP16, tag=f"x{name}",
                            name=f"xt_{name}_{i}", bufs=4,
                        )
                        nc.sync.dma_start(
                            out=xt,
                            in_=xin[i * 128 : (i + 1) * 128, 0 : nsc * 512],
                        )
                        for sc in range(nsc):
                            for ot in range(OT):
                                nc.tensor.matmul(
                                    acc[ot * nsc + sc],
                                    w_sb[:, i, ot * 128 : (ot + 1) * 128],
                                    xt[:, sc * 512 : (sc + 1) * 512],
                                    start=(i == 0),
                                    stop=(i == IT - 1),
                                )
                    for ot in range(OT):
                        for sc in range(nsc):
                            for half in range(2):
                                h = 2 * ot + half
                                p0 = half * 64
                                nc.vector.tensor_scalar_add(
                                    out=dst[0:64, h, sc * 512 : (sc + 1) * 512],
                                    in0=acc[ot * nsc + sc][p0 : p0 + 64, :],
                                    scalar1=t_bqk[p0 : p0 + 64, bidx, ot : ot + 1],
                                )

                # v natural: lhsT = x tile (stationary), rhs = wv (moving).
                # One accumulation group per psum bank (interleaving two
                # start/accumulate groups in one bank corrupts has_written).
                for rnd, sts in enumerate(vrounds):
                    sts = list(sts)
                    w = len(sts) * 128
                    vacc = [
                        ps_proj.tile([128, OC], F32, tag="proj", name=f"vacc_{rnd}_{n}")
                        for n in range(len(sts))
                    ]
                    for i in range(IT):
                        xt = xs.tile([128, w], FP16, tag="xv", name=f"xtv_{rnd}_{i}", bufs=3)
                        nc.sync.dma_start(
                            out=xt,
                            in_=xv[
                                i * 128 : (i + 1) * 128,
                                sts[0] * 128 : sts[0] * 128 + w,
                            ],
                        )
                        for n in range(len(sts)):
                            nc.tensor.matmul(
                                vacc[n],
                                xt[:, n * 128 : (n + 1) * 128],
                                t_wv[:, i, :],
                                start=(i == 0),
                                stop=(i == IT - 1),
                            )
                    for n, st in enumerate(sts):
                        nc.vector.tensor_add(
                            out=t_V[:, st, :, 0:DK],
                            in0=vacc[n].rearrange("p (h d) -> p h d", h=HPG),
                            in1=t_bv.rearrange("p (h d) -> p h d", h=HPG),
                        )

            # ---- phase B: attention (S.T layout) + interleaved phase C ----
            with (
                tc.tile_pool(name="ps_att", bufs=3, space=PSUM) as ps_att,
                tc.tile_pool(name="ps_o", bufs=3, space=PSUM) as ps_o,
                tc.tile_pool(name="ps_c", bufs=2, space=PSUM) as ps_c,
                tc.tile_pool(name="pb", bufs=4) as pb,
                tc.tile_pool(name="nrm", bufs=2) as nrm,
                tc.tile_pool(name="stg", bufs=4) as stg,
            ):
                for qc in range(SC):
                    q0 = qc * 512
                    nkb = min(4 * (qc + 1), kb_cap)
                    for pair in ((0, 1), (2, 3)):
                        o_ps = {
                            h: ps_o.tile(
                                [128, 512], F32, tag="ops", name=f"ops_{qc}_{h}"
                            )
                            for h in pair
                        }
                        for kb in range(nkb):
                            k0 = kb * 128
                            off = max(0, k0 - q0)
                            st = {}
                            # adjacent K=64 matmuls at row groups 0 / 64 can
                            # run concurrently in the PE array
                            for h in pair:
                                st[h] = ps_att.tile(
                                    [128, 512], F32, tag="st", name=f"st_{qc}_{h}_{kb}"
                                )
                                nc.tensor.matmul(
                                    st[h][:, off:512],
                                    t_kT[:, h, k0 : k0 + 128],
                                    t_qT[:, h, q0 + off : q0 + 512],
                                    start=True,
                                    stop=True,
                                )
                            for h in pair:
                                if k0 >= q0:
                                    nc.vector.tensor_add(
                                        out=st[h][:, off : off + 128],
                                        in0=st[h][:, off : off + 128],
                                        in1=t_causal,
                                    )
                                pt = pb.tile(
                                    [128, 512], FP16, tag="pt", name=f"pt_{qc}_{h}_{kb}"
                                )
                                nc.scalar.activation(
                                    out=pt[:, off:512],
                                    in_=st[h][:, off:512],
                                    func=Exp,
                                    bias=t_pad[:, kb : kb + 1],
                                    scale=1.0,
                                )
                                nc.tensor.matmul(
                                    o_ps[h][:, off:512],
                                    t_V[:, kb, h, :],
                                    pt[:, off:512],
                                    start=(kb == 0),
                                    stop=(kb == nkb - 1),
                                )
                        for h in pair:
                            ot, p0 = h // 2, (h % 2) * 64
                            t_l = nrm.tile([128, 512], F32, tag="l", name=f"l_{qc}_{h}")
                            nc.vector.tensor_copy(t_l[0:1, :], o_ps[h][DK : DK + 1, :])
                            t_r = nrm.tile([128, 512], F32, tag="r", name=f"r_{qc}_{h}")
                            nc.vector.reciprocal_approx_fast(t_r[0:1, :], t_l[0:1, :])
                            t_rb = nrm.tile([DK, 512], F32, tag="rb", name=f"rb_{qc}_{h}")
                            nc.gpsimd.partition_broadcast(t_rb, t_r[0:1, :])
                            nc.vector.tensor_mul(
                                t_OT[p0 : p0 + DK, ot, q0 : q0 + 512],
                                o_ps[h][0:DK, :],
                                t_rb,
                            )
                    # phase C for this 512-chunk of s (needs all 4 heads)
                    for dt_ in range(D // 128):
                        ops = ps_c.tile([128, 512], F32, tag="c", name=f"c_{qc}_{dt_}")
                        for j in range(OT):
                            nc.tensor.matmul(
                                ops,
                                t_wo[:, j, dt_ * 128 : (dt_ + 1) * 128],
                                t_OT[:, j, q0 : q0 + 512],
                                start=(j == 0),
                                stop=(j == OT - 1),
                            )
                        st_o = stg.tile([128, 512], F32, tag="s", name=f"so_{qc}_{dt_}")
                        nc.vector.tensor_copy(st_o, ops)
                        nc.sync.dma_start(
                            out=out_t[dt_ * 128 : (dt_ + 1) * 128, q0 : q0 + 512],
                            in_=st_o,
                        )
    nc.compile()
    return nc


def _get_nc(kb_cap):
    key = ("nc", kb_cap)
    if key not in _cache:
        _cache[key] = _build_nc(kb_cap)
    return _cache[key]


def kernel(
    query,
    key,
    value,
    Wq,
    bq,
    Wk,
    bk,
    Wv,
    bv,
    Wo,
    bo,
    attn_mask,
    key_padding_mask,
):
    import ml_dtypes
    from concourse import bass_utils

    query = np.asarray(query, dtype=np.float32)
    key = np.asarray(key, dtype=np.float32)
    value = np.asarray(value, dtype=np.float32)
    Wq = np.asarray(Wq, dtype=np.float32)
    bq = np.asarray(bq, dtype=np.float32)
    Wk = np.asarray(Wk, dtype=np.float32)
    bk = np.asarray(bk, dtype=np.float32)
    Wv = np.asarray(Wv, dtype=np.float32)
    bv = np.asarray(bv, dtype=np.float32)
    Wo = np.asarray(Wo, dtype=np.float32)
    bo = np.asarray(bo, dtype=np.float32)
    attn_mask = np.asarray(attn_mask)
    key_padding_mask = np.asarray(key_padding_mask)

    # this kernel hardcodes the causal structure of attn_mask
    expected = np.triu(np.ones((S, S), dtype=bool), k=1)
    assert np.array_equal(attn_mask, expected), "kernel assumes causal attn_mask"

    # number of 128-blocks that contain any valid (unpadded) key
    valid = ~key_padding_mask  # [B, S]
    kb_cap = 0
    for b in range(B):
        nz = np.nonzero(valid[b])[0]
        cap = (int(nz.max()) // 128 + 1) if nz.size else 1
        kb_cap = max(kb_cap, cap)

    scale = np.float32(1.0 / np.sqrt(DK))
    causal_tile = np.where(
        np.arange(128)[None, :] >= np.arange(128)[:, None], 0.0, NEG
    ).astype(np.float32)

    # per-batch transposed activations (shared by the batch's 4 cores)
    xq_b = [np.ascontiguousarray(query[:, b, :].T.astype(np.float16)) for b in range(B)]
    xk_b = [np.ascontiguousarray(key[:, b, :].T.astype(np.float16)) for b in range(B)]
    xv_b = [np.ascontiguousarray(value[:, b, :].T.astype(np.float16)) for b in range(B)]
    pad_b = [
        np.ascontiguousarray(
            np.where(key_padding_mask[b], NEG, 0.0)
            .astype(np.float32)
            .reshape(KB, 128)
            .T
        )
        for b in range(B)
    ]

    in_maps = []
    for c in range(N_CORES):
        b = c // GROUPS
        g = c % GROUPS
        o0 = g * OC
        osl = slice(o0, o0 + OC)
        bias_qk = np.stack(
            [
                (bq[osl] * scale).reshape(OT, 128).T,
                bk[osl].reshape(OT, 128).T,
            ],
            axis=1,
        ).astype(np.float32)  # [128, 2, OT]
        in_maps.append(
            {
                "xq": xq_b[b],
                "xk": xk_b[b],
                "xv": xv_b[b],
                "wq": np.ascontiguousarray((Wq[osl, :] * scale).T.astype(np.float16)),
                "wk": np.ascontiguousarray(Wk[osl, :].T.astype(np.float16)),
                "wv": np.ascontiguousarray(Wv[osl, :].T.astype(np.float16)),
                "wo": np.ascontiguousarray(Wo[:, osl].T).astype(np.float16),
                "bias_qk": np.ascontiguousarray(bias_qk),
                "bias_v": np.ascontiguousarray(bv[osl][None, :]),
                "pad": pad_b[b],
                "causal": causal_tile,
            }
        )

    res = bass_utils.run_bass_kernel_spmd(
        _get_nc(kb_cap), in_maps, core_ids=list(range(N_CORES))
    )
    _cache["last_res"] = res

    out = np.zeros((S, B, D), dtype=np.float32)
    for b in range(B):
        acc = np.zeros((D, S), dtype=np.float32)
        for g in range(GROUPS):
            acc += res.results[b * GROUPS + g]["out_t"]
        out[:, b, :] = acc.T + bo[None, :]
    return out


# revision 21
# speedup vs baseline: 1.2421x; 1.0201x over previous
"""Trainium2 Bass kernel for nn_MultiHeadAttention_59614146068609.

Sharding: 8 cores = 2 batches x 4 head-groups (4 heads each).
Each core projects q/k/v for its batch with its head-slice of Wq/Wk/Wv
(column-sharded), runs causal+padded attention for its 4 heads, and
applies its row-slice of Wo, producing a partial [D, S] output. The host
sums the 4 partials per batch and adds bo.

All matmuls run as float32r (2 cycles/row PE mode, ~1e-4 rel err).
Layout trick: scores are computed transposed (S.T[k, q], k on
partitions) so softmax sums come from an appended ones-column of V and
no on-chip transposes are needed anywhere.

The kernel is specialized at build time on kb_cap = number of 128-wide
key blocks that contain any unpadded key (derived from the runtime
key_padding_mask); fully padded key blocks contribute exactly zero
attention weight, so their projection/QK/exp/PV work is skipped.
"""

import numpy as np

S = 2048
B = 2
D = 1024
H = 16
DK = 64
N_CORES = 8
GROUPS = N_CORES // B          # head groups per batch = 4
HPG = H // GROUPS              # heads per group = 4
OC = HPG * DK                  # per-core projected dim = 256
OT = OC // 128                 # o-tiles per core = 2
IT = D // 128                  # contraction tiles = 8
SC = S // 512                  # sequence chunks of 512 = 4
KB = S // 128                  # k blocks of 128 = 16
NEG = -1e30

_cache = {}


def _build_nc(kb_cap):
    import concourse.bacc as bacc
    import concourse.bass as bass
    import concourse.mybir as mybir
    import concourse.tile as tile
    from concourse import library_config

    F32 = mybir.dt.float32
    F32R = mybir.dt.float32r
    FP16 = mybir.dt.float16
    Exp = mybir.ActivationFunctionType.Exp
    Identity = mybir.ActivationFunctionType.Identity
    PSUM = bass.MemorySpace.PSUM

    ksc = -(-kb_cap * 128 // 512)        # 512-chunks of k_T to project
    vrounds = [
        range(r * 8, min(kb_cap, (r + 1) * 8)) for r in range(-(-kb_cap // 8))
    ]

    nc = bacc.Bacc("TRN2", target_bir_lowering=False, debug=False)

    xq = nc.dram_tensor("xq", [D, S], FP16, kind="ExternalInput")
    xk = nc.dram_tensor("xk", [D, S], FP16, kind="ExternalInput")
    xv = nc.dram_tensor("xv", [D, S], FP16, kind="ExternalInput")
    wq = nc.dram_tensor("wq", [D, OC], FP16, kind="ExternalInput")
    wk = nc.dram_tensor("wk", [D, OC], FP16, kind="ExternalInput")
    wv = nc.dram_tensor("wv", [D, OC], FP16, kind="ExternalInput")
    wo = nc.dram_tensor("wo", [OC, D], FP16, kind="ExternalInput")
    bias_qk = nc.dram_tensor("bias_qk", [128, 2, OT], F32, kind="ExternalInput")
    bias_v = nc.dram_tensor("bias_v", [1, OC], F32, kind="ExternalInput")
    pad = nc.dram_tensor("pad", [128, KB], F32, kind="ExternalInput")
    causal = nc.dram_tensor("causal", [128, 128], FP16, kind="ExternalInput")
    out_t = nc.dram_tensor("out_t", [D, S], F32, kind="ExternalOutput")

    with tile.TileContext(nc) as tc, nc.allow_low_precision(
        reason="fp32r compute throughout; validated vs fp64 reference"
    ):
        with (
            tc.tile_pool(name="persist", bufs=1) as pp,
            tc.tile_pool(name="xs", bufs=6) as xs,
        ):
            nc.gpsimd.load_library(library_config.attn)

            # ---- persistent SBUF tensors ----
            t_wq = pp.tile([128, IT, OC], FP16)
            t_wk = pp.tile([128, IT, OC], FP16)
            t_wv = pp.tile([128, IT, OC], FP16)
            t_wo = pp.tile([128, OT, D], FP16)
            t_bqk = pp.tile([128, 2, OT], F32)
            t_bv = pp.tile([128, OC], F32)
            t_pad = pp.tile([128, KB], F32)
            t_causal = pp.tile([128, 128], FP16)
            t_qT = pp.tile([128, HPG, S], FP16)
            t_kT = pp.tile([128, HPG, ksc * 512], FP16)
            t_V = pp.tile([128, kb_cap, HPG, 128], FP16)
            t_OT = pp.tile([128, OT, S], FP16)

            nc.scalar.dma_start(out=t_wq, in_=wq[:].rearrange("(i p) o -> p i o", p=128))
            nc.scalar.dma_start(out=t_wk, in_=wk[:].rearrange("(i p) o -> p i o", p=128))
            nc.scalar.dma_start(out=t_wv, in_=wv[:].rearrange("(i p) o -> p i o", p=128))
            nc.scalar.dma_start(out=t_bqk, in_=bias_qk[:])
            nc.scalar.dma_start(out=t_pad, in_=pad[:])
            nc.scalar.dma_start(out=t_causal, in_=causal[:])
            # broadcast the v bias across partitions once (free dim = o)
            t_bv1 = pp.tile([1, OC], F32)
            nc.scalar.dma_start(out=t_bv1, in_=bias_v[:])
            nc.scalar.dma_start(out=t_wo, in_=wo[:].rearrange("(j p) d -> p j d", p=128))
            nc.gpsimd.partition_broadcast(t_bv, t_bv1)
            nc.gpsimd.memset(t_qT[64:128, :, :], 0)
            nc.gpsimd.memset(t_kT[64:128, :, :], 0)
            nc.gpsimd.memset(t_V[:], 0)
            nc.vector.memset(t_V[:, :, :, DK : DK + 1], 1.0)

            # ---- phase A: projections ----
            # q and k land transposed ([o, s], o on partitions); v lands
            # natural ([s, o], s on partitions) for the PV matmul.
            with tc.tile_pool(name="ps_proj", bufs=8, space=PSUM) as ps_proj:
                for name, xin, w_sb, nsc in (("q", xq, t_wq, SC), ("k", xk, t_wk, ksc)):
                    dst = t_qT if name == "q" else t_kT
                    bidx = 0 if name == "q" else 1
                    acc = [
                        ps_proj.tile(
                            [128, 512], F32, tag="proj", name=f"acc_{name}_{n}"
                        )
                        for n in range(OT * nsc)
                    ]
                    xts = []
                    for i in range(IT):
                        xt = xs.tile(
                            [128, nsc * 512], FP16, tag=f"x{name}",
                            name=f"xt_{name}_{i}", bufs=IT,
                        )
                        nc.sync.dma_start(
                            out=xt,
                            in_=xin[i * 128 : (i + 1) * 128, 0 : nsc * 512],
                        )
                        xts.append(xt)
                    for sc in range(nsc):
                        for ot in range(OT):
                            for i in range(IT):
                                nc.tensor.matmul(
                                    acc[ot * nsc + sc],
                                    w_sb[:, i, ot * 128 : (ot + 1) * 128],
                                    xts[i][:, sc * 512 : (sc + 1) * 512],
                                    start=(i == 0),
                                    stop=(i == IT - 1),
                                )
                    for ot in range(OT):
                        for sc in range(nsc):
                            for half in range(2):
                                h = 2 * ot + half
                                p0 = half * 64
                                nc.vector.tensor_scalar_add(
                                    out=dst[0:64, h, sc * 512 : (sc + 1) * 512],
                                    in0=acc[ot * nsc + sc][p0 : p0 + 64, :],
                                    scalar1=t_bqk[p0 : p0 + 64, bidx, ot : ot + 1],
                                )

                # v natural: lhsT = x tile (stationary), rhs = wv (moving).
                # One accumulation group per psum bank (interleaving two
                # start/accumulate groups in one bank corrupts has_written).
                for rnd, sts in enumerate(vrounds):
                    sts = list(sts)
                    w = len(sts) * 128
                    vacc = [
                        ps_proj.tile([128, OC], F32, tag="proj", name=f"vacc_{rnd}_{n}")
                        for n in range(len(sts))
                    ]
                    for i in range(IT):
                        xt = xs.tile([128, w], FP16, tag="xv", name=f"xtv_{rnd}_{i}", bufs=3)
                        nc.sync.dma_start(
                            out=xt,
                            in_=xv[
                                i * 128 : (i + 1) * 128,
                                sts[0] * 128 : sts[0] * 128 + w,
                            ],
                        )
                        for n in range(len(sts)):
                            nc.tensor.matmul(
                                vacc[n],
                                xt[:, n * 128 : (n + 1) * 128],
                                t_wv[:, i, :],
                                start=(i == 0),
                                stop=(i == IT - 1),
                            )
                    for n, st in enumerate(sts):
                        nc.vector.tensor_add(
                            out=t_V[:, st, :, 0:DK],
                            in0=vacc[n].rearrange("p (h d) -> p h d", h=HPG),
                            in1=t_bv.rearrange("p (h d) -> p h d", h=HPG),
                        )

            # ---- phase B: attention (S.T layout) + interleaved phase C ----
            with (
                tc.tile_pool(name="ps_att", bufs=3, space=PSUM) as ps_att,
                tc.tile_pool(name="ps_o", bufs=3, space=PSUM) as ps_o,
                tc.tile_pool(name="ps_c", bufs=2, space=PSUM) as ps_c,
                tc.tile_pool(name="pb", bufs=4) as pb,
                tc.tile_pool(name="nrm", bufs=2) as nrm,
                tc.tile_pool(name="stg", bufs=4) as stg,
            ):
                for qc in range(SC):
                    q0 = qc * 512
                    nkb = min(4 * (qc + 1), kb_cap)
                    for pair in ((0, 1), (2, 3)):
                        o_ps = {
                            h: ps_o.tile(
                                [128, 512], F32, tag="ops", name=f"ops_{qc}_{h}"
                            )
                            for h in pair
                        }
                        for kb in range(nkb):
                            k0 = kb * 128
                            off = max(0, k0 - q0)
                            st = {}
                            for h in pair:
                                st[h] = ps_att.tile(
                                    [128, 512], F32, tag="st", name=f"st_{qc}_{h}_{kb}"
                                )
                                nc.tensor.matmul(
                                    st[h][:, off:512],
                                    t_kT[:, h, k0 : k0 + 128],
                                    t_qT[:, h, q0 + off : q0 + 512],
                                    start=True,
                                    stop=True,
                                )
                            for h in pair:
                                if k0 >= q0:
                                    nc.vector.tensor_add(
                                        out=st[h][:, off : off + 128],
                                        in0=st[h][:, off : off + 128],
                                        in1=t_causal,
                                    )
                                pt = pb.tile(
                                    [128, 512], FP16, tag="pt", name=f"pt_{qc}_{h}_{kb}"
                                )
                                nc.scalar.activation(
                                    out=pt[:, off:512],
                                    in_=st[h][:, off:512],
                                    func=Exp,
                                    bias=t_pad[:, kb : kb + 1],
                                    scale=1.0,
                                )
                                nc.tensor.matmul(
                                    o_ps[h][:, off:512],
                                    t_V[:, kb, h, :],
                                    pt[:, off:512],
                                    start=(kb == 0),
                                    stop=(kb == nkb - 1),
                                )
                        for h in pair:
                            ot, p0 = h // 2, (h % 2) * 64
                            t_l = nrm.tile([128, 512], F32, tag="l", name=f"l_{qc}_{h}")
                            nc.vector.tensor_copy(
                                t_l[0:1, :], o_ps[h][DK : DK + 1, :]
                            )
                            t_r = nrm.tile([128, 512], F32, tag="r", name=f"r_{qc}_{h}")
                            nc.vector.reciprocal_approx_fast(t_r[0:1, :], t_l[0:1, :])
                            t_rb = nrm.tile([DK, 512], F32, tag="rb", name=f"rb_{qc}_{h}")
                            nc.gpsimd.partition_broadcast(t_rb, t_r[0:1, :])
                            nc.vector.tensor_mul(
                                t_OT[p0 : p0 + DK, ot, q0 : q0 + 512],
                                o_ps[h][0:DK, :],
                                t_rb,
                            )
                    # phase C for this 512-chunk of s (needs all 4 heads)
                    for dt_ in range(D // 128):
                        ops = ps_c.tile([128, 512], F32, tag="c", name=f"c_{qc}_{dt_}")
                        for j in range(OT):
                            nc.tensor.matmul(
                                ops,
                                t_wo[:, j, dt_ * 128 : (dt_ + 1) * 128],
                                t_OT[:, j, q0 : q0 + 512],
                                start=(j == 0),
                                stop=(j == OT - 1),
                            )
                        st_o = stg.tile([128, 512], F32, tag="s", name=f"so_{qc}_{dt_}")
                        nc.vector.tensor_copy(st_o, ops)
                        nc.sync.dma_start(
                            out=out_t[dt_ * 128 : (dt_ + 1) * 128, q0 : q0 + 512],
                            in_=st_o,
                        )
    nc.compile()
    return nc


def _get_nc(kb_cap):
    key = ("nc", kb_cap)
    if key not in _cache:
        _cache[key] = _build_nc(kb_cap)
    return _cache[key]


def kernel(
    query,
    key,
    value,
    Wq,
    bq,
    Wk,
    bk,
    Wv,
    bv,
    Wo,
    bo,
    attn_mask,
    key_padding_mask,
):
    import ml_dtypes
    from concourse import bass_utils

    query = np.asarray(query, dtype=np.float32)
    key = np.asarray(key, dtype=np.float32)
    value = np.asarray(value, dtype=np.float32)
    Wq = np.asarray(Wq, dtype=np.float32)
    bq = np.asarray(bq, dtype=np.float32)
    Wk = np.asarray(Wk, dtype=np.float32)
    bk = np.asarray(bk, dtype=np.float32)
    Wv = np.asarray(Wv, dtype=np.float32)
    bv = np.asarray(bv, dtype=np.float32)
    Wo = np.asarray(Wo, dtype=np.float32)
    bo = np.asarray(bo, dtype=np.float32)
    attn_mask = np.asarray(attn_mask)
    key_padding_mask = np.asarray(key_padding_mask)

    # this kernel hardcodes the causal structure of attn_mask
    expected = np.triu(np.ones((S, S), dtype=bool), k=1)
    assert np.array_equal(attn_mask, expected), "kernel assumes causal attn_mask"

    # number of 128-blocks that contain any valid (unpadded) key
    valid = ~key_padding_mask  # [B, S]
    kb_cap = 0
    for b in range(B):
        nz = np.nonzero(valid[b])[0]
        cap = (int(nz.max()) // 128 + 1) if nz.size else 1
        kb_cap = max(kb_cap, cap)

    scale = np.float32(1.0 / np.sqrt(DK))
    causal_tile = np.where(
        np.arange(128)[None, :] >= np.arange(128)[:, None], 0.0, -60000.0
    ).astype(np.float16)

    # per-batch transposed activations (shared by the batch's 4 cores)
    xq_b = [np.ascontiguousarray(query[:, b, :].T.astype(np.float16)) for b in range(B)]
    xk_b = [np.ascontiguousarray(key[:, b, :].T.astype(np.float16)) for b in range(B)]
    xv_b = [np.ascontiguousarray(value[:, b, :].T.astype(np.float16)) for b in range(B)]
    pad_b = [
        np.ascontiguousarray(
            np.where(key_padding_mask[b], NEG, 0.0)
            .astype(np.float32)
            .reshape(KB, 128)
            .T
        )
        for b in range(B)
    ]

    in_maps = []
    for c in range(N_CORES):
        b = c // GROUPS
        g = c % GROUPS
        o0 = g * OC
        osl = slice(o0, o0 + OC)
        bias_qk = np.stack(
            [
                (bq[osl] * scale).reshape(OT, 128).T,
                bk[osl].reshape(OT, 128).T,
            ],
            axis=1,
        ).astype(np.float32)  # [128, 2, OT]
        in_maps.append(
            {
                "xq": xq_b[b],
                "xk": xk_b[b],
                "xv": xv_b[b],
                "wq": np.ascontiguousarray((Wq[osl, :] * scale).T.astype(np.float16)),
                "wk": np.ascontiguousarray(Wk[osl, :].T.astype(np.float16)),
                "wv": np.ascontiguousarray(Wv[osl, :].T.astype(np.float16)),
                "wo": np.ascontiguousarray(Wo[:, osl].T).astype(np.float16),
                "bias_qk": np.ascontiguousarray(bias_qk),
                "bias_v": np.ascontiguousarray(bv[osl][None, :]),
                "pad": pad_b[b],
                "causal": causal_tile,
            }
        )

    res = bass_utils.run_bass_kernel_spmd(
        _get_nc(kb_cap), in_maps, core_ids=list(range(N_CORES))
    )
    _cache["last_res"] = res

    out = np.zeros((S, B, D), dtype=np.float32)
    for b in range(B):
        acc = np.zeros((D, S), dtype=np.float32)
        for g in range(GROUPS):
            acc += res.results[b * GROUPS + g]["out_t"]
        out[:, b, :] = acc.T + bo[None, :]
    return out
